# revision 1
# baseline (speedup 1.0000x reference)
"""Trainium2 8-core MoE layer kernel (expert-parallel, Bass/Tile).

Contract: kernel(**inputs) takes the full unsharded numpy inputs of the
MoE reference (hidden_states, router_w, w1, b1, w2, b2) and returns the
full [2, 1024, 2048] float32 output. Internally shards across 8
NeuronCores: one expert per core, replicated FFN weights in bf16,
sharded fp32 router with an AllGather of routing decisions, dispatch via
one-hot permutation matmul, and a column-chunked AllGather + indirect-
gather combine overlapped with the second FFN matmul.
"""
import numpy as np
import ml_dtypes

import concourse.bass as bass
import concourse.mybir as mybir
import concourse.tile as tile

_PATCH_DOC = """Patch TileContext._drain_and_barrier: the stock version stuffs every
outstanding semaphore wait onto one SP Drain instruction; the installed
walrus rejects >1 sync wait per non-EventSemaphore instruction
("Too many sync wait commands"). Split the waits across a chain of SP
nops, then drain/barrier as before."""
import concourse.tile as tile_mod
from concourse.vector_clock import ScopedClock


def _patched_drain_and_barrier(self, tick_clock, wait_clock):
    nc = self.nc
    carrier = nc.sync.nop(nofuse=True, hint="drain_wait_carrier")
    wait_clock.add_sem_waits(
        carrier.ins, ScopedClock({None: tick_clock.global_clock})
    )
    waits = list(carrier.ins.sync_info.on_wait)
    if len(waits) > 1:
        carrier.ins.sync_info.on_wait = waits[:1]
        import bass_rust as _br
        for w in waits[1:]:
            extra = nc.sync.nop(nofuse=True, hint="drain_wait_carrier")
            extra.ins.sync_info = _br.SyncInfo(on_wait=[w], on_update=[])

    nc.sync.drain()
    nc.all_engine_barrier()
    assert self.sems is not None
    popped = nc._tile_sem_poison_stack.pop()
    assert popped is self._sem_poison
    nc.clear_and_free_semaphores(list(self.sems.allocated().values()))
    nc.all_engine_barrier()


def apply():
    tile_mod.TileContext._drain_and_barrier = _patched_drain_and_barrier


import concourse.mybir as mybir
import bass_rust as _br


def split_multi_waits(nc):
    """Walrus in this container accepts at most ONE sync wait per
    instruction. Hoist extra waits onto same-engine NoOps inserted
    immediately before the offending instruction."""
    ctr = 0
    for f in nc.m.functions:
        for b in f.blocks:
            insts = b.instructions
            need = any(
                inst.sync_info is not None and len(inst.sync_info.on_wait) > 1
                for inst in insts
            )
            if not need:
                continue
            out = []
            for inst in insts:
                si = inst.sync_info
                if si is not None and len(si.on_wait) > 1:
                    waits = list(si.on_wait)
                    for w in waits[:-1]:
                        nop = mybir.InstNoOp(name=f"I-wsplit-{ctr}", ins=[], outs=[])
                        ctr += 1
                        nop.engine = inst.engine
                        nop.sync_info = _br.SyncInfo(on_wait=[w], on_update=[])
                        out.append(nop)
                    si.on_wait = waits[-1:]
                out.append(inst)
            b.instructions = out
    return ctr







E, TOPK, CAP, H, F, N, NCORES = 8, 2, 512, 2048, 8192, 2048, 8
S = CAP
TT = N // 128                # 16 token tiles
HT = H // 128                # 16 hidden tiles
FT = F // 128                # 64 ffn tiles
NQ = 8                       # AllGather column chunks
QH = H // NQ                 # 256
TOKC = N // NCORES           # 256

f32 = mybir.dt.float32
f16 = mybir.dt.float16
bf16 = mybir.dt.bfloat16
i32 = mybir.dt.int32
AOP = mybir.AluOpType
AFT = mybir.ActivationFunctionType
AX = mybir.AxisListType


def build_moe(nc: bass.Bass):
    xtm = nc.dram_tensor("xtm", [2, 128, H], f32, kind="ExternalInput")
    xr = nc.dram_tensor("xr", [N, H], bf16, kind="ExternalInput")
    rwT = nc.dram_tensor("rwT", [H, E], f32, kind="ExternalInput")
    w1T = nc.dram_tensor("w1tt", [FT, 128, HT * 128], bf16, kind="ExternalInput")
    w2T = nc.dram_tensor("w2T", [F, H], bf16, kind="ExternalInput")
    b1t = nc.dram_tensor("b1t", [128, FT], f32, kind="ExternalInput")
    b2r = nc.dram_tensor("b2r", [1, H], f32, kind="ExternalInput")
    cid = nc.dram_tensor("cid", [1, 1], f32, kind="ExternalInput")
    out = nc.dram_tensor("out", [TOKC, H], f32, kind="ExternalOutput")

    rloc = nc.dram_tensor("rloc", [TOKC, 4], f32)
    rall = nc.dram_tensor("rall", [N, 4], f32, addr_space="Shared")
    posd = nc.dram_tensor("posd", [1, 2 * N], f32)
    cmbd = nc.dram_tensor("cmbd", [TT, 128, 5], f32)
    crow = nc.dram_tensor("crow", [2, 128 * 5], f32)
    hcq = [nc.dram_tensor(f"hc{q}", [S, QH], bf16) for q in range(NQ)]
    hga = [nc.dram_tensor(f"hg{q}", [E * CAP + 1, QH], bf16, addr_space="Shared")
           for q in range(NQ)]

    with tile.TileContext(nc, num_cores=NCORES) as tc:
        with tc.tile_pool(name="persist", bufs=1) as persist:
            _body(nc, tc, persist, xtm, xr, rwT, w1T, w2T, b1t, b2r, cid, out,
                  rloc, rall, posd, cmbd, crow, hcq, hga)
    return nc


def _body(nc, tc, persist, xtm, xr, rwT, w1T, w2T, b1t, b2r, cid, out,
          rloc, rall, posd, cmbd, crow, hcq, hga):
    # ---- persistent tiles ----
    e0a = persist.tile([128, TT], f32, tag="e0a")
    e1a = persist.tile([128, TT], f32, tag="e1a")
    p0a = persist.tile([128, TT], f32, tag="p0a")
    p1a = persist.tile([128, TT], f32, tag="p1a")
    pos0a = persist.tile([128, TT], f32, tag="pos0a")
    pos1a = persist.tile([128, TT], f32, tag="pos1a")
    keep0 = persist.tile([128, TT], f32, tag="keep0")
    keep1 = persist.tile([128, TT], f32, tag="keep1")
    cmb = persist.tile([128, TT * 5], f32, tag="cmb")
    b2b = persist.tile([128, H], f32, tag="b2b")
    cidb = persist.tile([128, 1], f32, tag="cidb")
    zrow = persist.tile([1, QH], bf16, tag="zrow")
    iota512 = persist.tile([128, S], f32, tag="iota512")
    b1sb = persist.tile([128, FT], f32, tag="b1sb")
    rws = persist.tile([128, HT * E], f32, tag="rws")

    nc.gpsimd.dma_start(out=b2b[:], in_=b2r[0:1, :].partition_broadcast(128).opt())
    nc.gpsimd.dma_start(out=cidb[:], in_=cid[0:1, :].partition_broadcast(128).opt())
    nc.gpsimd.dma_start(out=b1sb[:], in_=b1t[:, :])
    nc.vector.memset(zrow[:], 0.0)
    for q in range(NQ):
        nc.gpsimd.dma_start(out=hga[q][E * CAP:E * CAP + 1, :], in_=zrow[:])
    with tc.tile_pool(name="iota_tmp", bufs=1) as it_p:
        iota512i = it_p.tile([128, S], i32, tag="iota512i")
        nc.gpsimd.iota(iota512i[:], pattern=[[1, S]], base=0, channel_multiplier=0)
        nc.vector.tensor_copy(out=iota512[:], in_=iota512i[:])
    nc.scalar.dma_start(
        out=rws[:].rearrange("p (c e) -> p c e", e=E),
        in_=rwT[:, :].rearrange("(c p) e -> p c e", p=128))

    # resident dispatch lhsT tiles (plain row-major x, prefetched early)
    xr_pool = tc.tile_pool(name="xr_res", bufs=1)
    xr_res = xr_pool.__enter__()
    xrt = []
    for tt in range(TT):
        xt = xr_res.tile([128, H], bf16, tag=f"xr_{tt}")
        nc.sync.dma_start(out=xt[:], in_=xr[tt * 128:(tt + 1) * 128, :])
        xrt.append(xt)

    # ============ Phase R: sharded router (own 256 tokens, fp32) ============
    with (tc.tile_pool(name="r_x", bufs=2) as r_x,
          tc.tile_pool(name="r_ps", bufs=2, space="PSUM") as r_ps,
          tc.tile_pool(name="r_sb", bufs=2) as r_sb):
        for tt2 in range(2):
            xt_t = r_x.tile([128, H], f32, tag="xt_t")
            for qq in range(2):
                nc.scalar.dma_start(
                    out=xt_t[:, qq * (H // 2):(qq + 1) * (H // 2)],
                    in_=xtm[tt2, :, qq * (H // 2):(qq + 1) * (H // 2)])
            ps = r_ps.tile([128, E], f32, tag="r_ps")
            for hc in range(HT):
                nc.tensor.matmul(
                    out=ps[:], lhsT=xt_t[:, hc * 128:(hc + 1) * 128],
                    rhs=rws[:, hc * E:(hc + 1) * E],
                    start=(hc == 0), stop=(hc == HT - 1))
            lsb = r_sb.tile([128, E], f32, tag="lsb")
            nc.vector.tensor_copy(out=lsb[:], in_=ps[:])
            mx = r_sb.tile([128, 1], f32, tag="mx")
            nc.vector.tensor_reduce(out=mx[:], in_=lsb[:], op=AOP.max, axis=AX.X)
            nm = r_sb.tile([128, 1], f32, tag="nm")
            nc.vector.tensor_scalar_mul(nm[:], mx[:], -1.0)
            ex = r_sb.tile([128, E], f32, tag="ex")
            ssum = r_sb.tile([128, 1], f32, tag="ssum")
            nc.scalar.activation(out=ex[:], in_=lsb[:], func=AFT.Exp,
                                 bias=nm[:], scale=1.0, accum_out=ssum[:])
            rcp = r_sb.tile([128, 1], f32, tag="rcp")
            nc.vector.reciprocal(out=rcp[:], in_=ssum[:])
            pr = r_sb.tile([128, E], f32, tag="pr")
            nc.vector.tensor_scalar_mul(pr[:], ex[:], rcp[:])
            mx8 = r_sb.tile([128, 8], f32, tag="mx8")
            ix8 = r_sb.tile([128, 8], mybir.dt.uint32, tag="ix8")
            nc.vector.max_with_indices(out_max=mx8[:], out_indices=ix8[:],
                                       in_=pr[:])
            rv = r_sb.tile([128, 4], f32, tag="rv")
            nc.vector.tensor_copy(out=rv[:, 0:1], in_=ix8[:, 0:1])
            nc.vector.tensor_copy(out=rv[:, 1:2], in_=ix8[:, 1:2])
            nc.vector.tensor_copy(out=rv[:, 2:3], in_=mx8[:, 0:1])
            nc.vector.tensor_copy(out=rv[:, 3:4], in_=mx8[:, 1:2])
            nc.scalar.dma_start(out=rloc[tt2 * 128:(tt2 + 1) * 128, :], in_=rv[:])
        nc.gpsimd.collective_compute(
            "AllGather", AOP.bypass,
            replica_groups=[list(range(NCORES))],
            ins=[rloc[:, :].opt()],
            outs=[rall[:, :].opt()])

    # read back full routing (reference token order)
    nc.scalar.dma_start(
        out=e0a[:], in_=rall[:, 0:1].rearrange("(t p) a -> (a p) t", p=128))
    nc.scalar.dma_start(
        out=e1a[:], in_=rall[:, 1:2].rearrange("(t p) a -> (a p) t", p=128))
    nc.scalar.dma_start(
        out=p0a[:], in_=rall[:, 2:3].rearrange("(t p) a -> (a p) t", p=128))
    nc.scalar.dma_start(
        out=p1a[:], in_=rall[:, 3:4].rearrange("(t p) a -> (a p) t", p=128))

    # ============ Phase S: one-hot + scan (E-major, fp16) ============
    with (tc.tile_pool(name="scan", bufs=1) as sc,
          tc.tile_pool(name="s_ps", bufs=2, space="PSUM") as s_ps):
        iop8 = sc.tile([E, 1], i32, tag="iop8")
        iop8f = sc.tile([E, 1], f32, tag="iop8f")
        nc.gpsimd.iota(iop8[:], pattern=[[0, 1]], base=0, channel_multiplier=1)
        nc.vector.tensor_copy(out=iop8f[:], in_=iop8[:])
        e0b = sc.tile([E, N], f32, tag="e0b")
        e1b = sc.tile([E, N], f32, tag="e1b")
        nc.scalar.dma_start(
            out=e0b[:],
            in_=rall[:, 0:1].rearrange("n a -> a n").partition_broadcast(E).opt())
        nc.scalar.dma_start(
            out=e1b[:],
            in_=rall[:, 1:2].rearrange("n a -> a n").partition_broadcast(E).opt())
        ohcat = sc.tile([E, 2 * N], f32, tag="ohcat")
        nc.vector.tensor_scalar(out=ohcat[:, :N], in0=e0b[:], scalar1=iop8f[:],
                                scalar2=None, op0=AOP.is_equal)
        nc.vector.tensor_scalar(out=ohcat[:, N:], in0=e1b[:], scalar1=iop8f[:],
                                scalar2=None, op0=AOP.is_equal)
        ones2n = sc.tile([E, 2 * N], f32, tag="ones2n")
        nc.vector.memset(ones2n[:], 1.0)
        cum = sc.tile([E, 2 * N], f32, tag="cum")
        nc.vector.tensor_tensor_scan(out=cum[:], data0=ones2n[:], data1=ohcat[:],
                                     initial=0.0, op0=AOP.mult, op1=AOP.add)
        ohcum = sc.tile([E, 2 * N], f16, tag="ohcum")
        nc.vector.tensor_tensor(out=ohcum[:], in0=ohcat[:], in1=cum[:],
                                op=AOP.mult)
        ones8 = sc.tile([E, 1], f16, tag="ones8")
        nc.vector.memset(ones8[:], 1.0)
        posrow = sc.tile([1, 2 * N], f32, tag="posrow")
        for ch in range(2 * N // 512):
            pps = s_ps.tile([1, 512], f32, tag="pps")
            nc.tensor.matmul(out=pps[:], lhsT=ones8[:],
                             rhs=ohcum[:, ch * 512:(ch + 1) * 512],
                             start=True, stop=True)
            nc.vector.tensor_scalar_add(
                posrow[:, ch * 512:(ch + 1) * 512], pps[:], -1.0)
        nc.scalar.dma_start(out=posd[:, :], in_=posrow[:])

    nc.scalar.dma_start(
        out=pos0a[:], in_=posd[0:1, 0:N].rearrange("a (t p) -> (a p) t", p=128))
    nc.scalar.dma_start(
        out=pos1a[:], in_=posd[0:1, N:2 * N].rearrange("a (t p) -> (a p) t", p=128))

    # ============ Phase I: token-major index math ============
    cmb_v = cmb[:].rearrange("p (t j) -> p t j", j=5)
    nc.vector.tensor_scalar(out=keep0[:], in0=pos0a[:], scalar1=float(CAP),
                            scalar2=None, op0=AOP.is_lt)
    nc.vector.tensor_scalar(out=keep1[:], in0=pos1a[:], scalar1=float(CAP),
                            scalar2=None, op0=AOP.is_lt)
    ECAPF = float(E * CAP)
    with tc.tile_pool(name="imath", bufs=1) as im:
        d0 = im.tile([128, TT], f32, tag="d0")
        d1 = im.tile([128, TT], f32, tag="d1")
        nc.vector.scalar_tensor_tensor(out=d0[:], in0=e0a[:], scalar=float(CAP),
                                       in1=pos0a[:], op0=AOP.mult, op1=AOP.add)
        nc.vector.scalar_tensor_tensor(out=d1[:], in0=e1a[:], scalar=float(CAP),
                                       in1=pos1a[:], op0=AOP.mult, op1=AOP.add)
        t0 = im.tile([128, TT], f32, tag="t0")
        t1 = im.tile([128, TT], f32, tag="t1")
        nc.vector.scalar_tensor_tensor(out=t0[:], in0=d0[:], scalar=-ECAPF,
                                       in1=keep0[:], op0=AOP.add, op1=AOP.mult)
        nc.vector.scalar_tensor_tensor(out=t1[:], in0=d1[:], scalar=-ECAPF,
                                       in1=keep1[:], op0=AOP.add, op1=AOP.mult)
        nc.vector.tensor_scalar_add(cmb_v[:, :, 0].opt(), t0[:], ECAPF)
        nc.vector.tensor_scalar_add(cmb_v[:, :, 1].opt(), t1[:], ECAPF)
        nc.vector.tensor_tensor(out=cmb_v[:, :, 2].opt(), in0=p0a[:],
                                in1=keep0[:], op=AOP.mult)
        nc.vector.tensor_tensor(out=cmb_v[:, :, 3].opt(), in0=p1a[:],
                                in1=keep1[:], op=AOP.mult)
        nc.vector.tensor_tensor(out=cmb_v[:, :, 4].opt(),
                                in0=cmb_v[:, :, 2].opt(),
                                in1=cmb_v[:, :, 3].opt(), op=AOP.add)
    nc.scalar.dma_start(out=cmbd[:, :, :].rearrange("t p j -> p t j"),
                        in_=cmb[:].rearrange("p (t j) -> p t j", j=5))

    # ============ Phase C-prep ============
    ct_tiles = []
    d0i_tiles = []
    d1i_tiles = []
    with tc.tile_pool(name="cprep", bufs=1) as cp:
        iop2 = cp.tile([2, 1], i32, tag="iop2")
        iop2f = cp.tile([2, 1], f32, tag="iop2f")
        nc.gpsimd.iota(iop2[:], pattern=[[0, 1]], base=0, channel_multiplier=1)
        nc.vector.tensor_copy(out=iop2f[:], in_=iop2[:])
        cidb2 = cp.tile([2, 1], f32, tag="cidb2")
        nc.scalar.dma_start(out=cidb2[:],
                            in_=cid[0:1, :].partition_broadcast(2).opt())
        idx2f = cp.tile([2, 1], f32, tag="idx2f")
        nc.vector.scalar_tensor_tensor(out=idx2f[:], in0=cidb2[:], scalar=2.0,
                                       in1=iop2f[:], op0=AOP.mult, op1=AOP.add)
        idx2i = cp.tile([2, 1], i32, tag="idx2i")
        nc.vector.tensor_copy(out=idx2i[:], in_=idx2f[:])
        cmb2 = cp.tile([2, 128 * 5], f32, tag="cmb2")
        nc.gpsimd.indirect_dma_start(
            out=cmb2[:], out_offset=None,
            in_=cmbd[:, :, :].rearrange("t p j -> t (p j)"),
            in_offset=bass.IndirectOffsetOnAxis(ap=idx2i[:, :1], axis=0))
        for tt2 in range(2):
            nc.scalar.dma_start(out=crow[tt2:tt2 + 1, :],
                                in_=cmb2[tt2:tt2 + 1, :])
        for tt2 in range(2):
            ct = persist.tile([128, 5], f32, tag=f"ct_{tt2}")
            nc.scalar.dma_start(out=ct[:],
                                in_=crow[tt2:tt2 + 1, :].rearrange(
                                    "a (p j) -> (a p) j", p=128))
            d0i = persist.tile([128, 1], i32, tag=f"d0i_{tt2}")
            d1i = persist.tile([128, 1], i32, tag=f"d1i_{tt2}")
            nc.vector.tensor_copy(out=d0i[:], in_=ct[:, 0:1])
            nc.vector.tensor_copy(out=d1i[:], in_=ct[:, 1:2])
            ct_tiles.append(ct)
            d0i_tiles.append(d0i)
            d1i_tiles.append(d1i)

    # ============ Phase D: P_c build + dispatch matmul ============
    xcT = []
    with tc.tile_pool(name="xc", bufs=1) as xc_pool:
        with (tc.tile_pool(name="dp", bufs=1) as dp,
              tc.tile_pool(name="dp2", bufs=2) as dp2,
              tc.tile_pool(name="d_ps", bufs=1, space="PSUM") as d_ps):
            ptiles = []
            for tt in range(TT):
                m0 = dp2.tile([128, 1], f32, tag="m0")
                m1 = dp2.tile([128, 1], f32, tag="m1")
                nc.vector.tensor_scalar(out=m0[:], in0=e0a[:, tt:tt + 1],
                                        scalar1=cidb[:], scalar2=None,
                                        op0=AOP.is_equal)
                nc.vector.tensor_scalar(out=m1[:], in0=e1a[:, tt:tt + 1],
                                        scalar1=cidb[:], scalar2=None,
                                        op0=AOP.is_equal)
                nc.vector.tensor_tensor(out=m0[:], in0=m0[:],
                                        in1=keep0[:, tt:tt + 1], op=AOP.mult)
                nc.vector.tensor_tensor(out=m1[:], in0=m1[:],
                                        in1=keep1[:, tt:tt + 1], op=AOP.mult)
                ps0 = dp2.tile([128, 1], f32, tag="ps0")
                ps1 = dp2.tile([128, 1], f32, tag="ps1")
                nc.vector.scalar_tensor_tensor(out=ps0[:],
                                               in0=pos0a[:, tt:tt + 1],
                                               scalar=1.0, in1=m0[:],
                                               op0=AOP.add, op1=AOP.mult)
                nc.vector.scalar_tensor_tensor(out=ps1[:],
                                               in0=pos1a[:, tt:tt + 1],
                                               scalar=1.0, in1=m1[:],
                                               op0=AOP.add, op1=AOP.mult)
                nc.vector.tensor_scalar_add(ps0[:], ps0[:], -1.0)
                nc.vector.tensor_scalar_add(ps1[:], ps1[:], -1.0)
                oh1 = dp2.tile([128, S], bf16, tag="oh1")
                nc.vector.tensor_scalar(out=oh1[:], in0=iota512[:],
                                        scalar1=ps1[:], scalar2=None,
                                        op0=AOP.is_equal)
                ptile = dp.tile([128, S], bf16, tag=f"pt_{tt}")
                nc.vector.scalar_tensor_tensor(out=ptile[:], in0=iota512[:],
                                               scalar=ps0[:], in1=oh1[:],
                                               op0=AOP.is_equal, op1=AOP.add)
                ptiles.append(ptile)

            for half in range(2):
                pd = []
                for hti in range(8):
                    pd_t = d_ps.tile([128, S], f32, tag=f"d_ps_{hti}")
                    pd.append(pd_t)
                for tt in range(TT):
                    for hti in range(8):
                        ht = half * 8 + hti
                        nc.tensor.matmul(
                            out=pd[hti][:],
                            lhsT=xrt[tt][:, ht * 128:(ht + 1) * 128],
                            rhs=ptiles[tt][:],
                            start=(tt == 0), stop=(tt == TT - 1))
                for hti in range(8):
                    xt = xc_pool.tile([128, S], bf16, tag=f"xcT_{half * 8 + hti}")
                    nc.scalar.copy(out=xt[:], in_=pd[hti][:])
                    xcT.append(xt)

        # ============ Phase F1 ============
        with tc.tile_pool(name="g", bufs=1) as g_pool:
            g = []
            with (tc.tile_pool(name="f1_w", bufs=4) as f1_w,
                  tc.tile_pool(name="f1_ps", bufs=2, space="PSUM") as f1_ps):
                for ft in range(FT):
                    w1_t = f1_w.tile([128, HT * 128], bf16, tag="w1_t")
                    QW = HT * 128 // 2
                    for qq in range(2):
                        nc.sync.dma_start(
                            out=w1_t[:, qq * QW:(qq + 1) * QW],
                            in_=w1T[ft, :, qq * QW:(qq + 1) * QW])
                    ps = f1_ps.tile([128, S], f32, tag="f1_ps")
                    for hc in range(HT):
                        nc.tensor.matmul(
                            out=ps[:], lhsT=w1_t[:, hc * 128:(hc + 1) * 128],
                            rhs=xcT[hc][:],
                            start=(hc == 0), stop=(hc == HT - 1))
                    gt = g_pool.tile([128, S], bf16, tag=f"g_{ft}")
                    nc.scalar.activation(out=gt[:], in_=ps[:], func=AFT.Gelu,
                                         bias=b1sb[:, ft:ft + 1], scale=1.0)
                    g.append(gt)

            # ============ Phase F2 + AllGather + combine per column chunk ====
            with (tc.tile_pool(name="f2_w", bufs=8) as f2_w,
                  tc.tile_pool(name="f2_ps", bufs=2, space="PSUM") as f2_ps,
                  tc.tile_pool(name="f2_o", bufs=4) as f2_o,
                  tc.tile_pool(name="cbp", bufs=2) as cbp):
                for q in range(NQ):
                    psq = []
                    for mt in range(4):
                        psq_t = f2_ps.tile([128, QH], f32, tag=f"f2_ps_{mt}")
                        psq.append(psq_t)
                    for fc in range(FT):
                        w2_t = f2_w.tile([128, QH], bf16, tag="w2_t")
                        eng = nc.sync if (fc % 2 == 0) else nc.scalar
                        eng.dma_start(
                            out=w2_t[:],
                            in_=w2T[fc * 128:(fc + 1) * 128,
                                    q * QH:(q + 1) * QH])
                        for mt in range(4):
                            nc.tensor.matmul(
                                out=psq[mt][:],
                                lhsT=g[fc][:, mt * 128:(mt + 1) * 128],
                                rhs=w2_t[:],
                                start=(fc == 0), stop=(fc == FT - 1))
                    for mt in range(4):
                        ho = f2_o.tile([128, QH], bf16, tag="ho")
                        nc.scalar.copy(out=ho[:], in_=psq[mt][:])
                        nc.sync.dma_start(out=hcq[q][mt * 128:(mt + 1) * 128, :],
                                          in_=ho[:])
                    nc.gpsimd.collective_compute(
                        "AllGather", AOP.bypass,
                        replica_groups=[list(range(NCORES))],
                        ins=[hcq[q][:, :].opt()],
                        outs=[hga[q][0:E * CAP, :].opt()])
                    for tt2 in range(2):
                        ct = ct_tiles[tt2]
                        g0 = cbp.tile([128, QH], bf16, tag="g0")
                        g1 = cbp.tile([128, QH], bf16, tag="g1")
                        nc.gpsimd.indirect_dma_start(
                            out=g0[:], out_offset=None, in_=hga[q][:, :],
                            in_offset=bass.IndirectOffsetOnAxis(
                                ap=d0i_tiles[tt2][:, :1], axis=0))
                        nc.gpsimd.indirect_dma_start(
                            out=g1[:], out_offset=None, in_=hga[q][:, :],
                            in_offset=bass.IndirectOffsetOnAxis(
                                ap=d1i_tiles[tt2][:, :1], axis=0))
                        a0 = cbp.tile([128, QH], f32, tag="a0")
                        nc.vector.tensor_scalar_mul(a0[:], g0[:], ct[:, 2:3])
                        a1 = cbp.tile([128, QH], f32, tag="a1")
                        nc.vector.scalar_tensor_tensor(
                            out=a1[:], in0=g1[:], scalar=ct[:, 3:4], in1=a0[:],
                            op0=AOP.mult, op1=AOP.add)
                        a2 = cbp.tile([128, QH], f32, tag="a2")
                        nc.vector.scalar_tensor_tensor(
                            out=a2[:], in0=b2b[:, q * QH:(q + 1) * QH],
                            scalar=ct[:, 4:5], in1=a1[:],
                            op0=AOP.mult, op1=AOP.add)
                        nc.gpsimd.dma_start(
                            out=out[tt2 * 128:(tt2 + 1) * 128,
                                    q * QH:(q + 1) * QH],
                            in_=a2[:])
    xr_pool.__exit__(None, None, None)



# ======================== host-side glue ========================

_CACHE = {}


def _prep_inputs(hidden_states, router_w, w1, b1, w2, b2):
    x = np.asarray(hidden_states, np.float32).reshape(-1, H)
    xT = x.T
    xr = x.astype(ml_dtypes.bfloat16)
    w1Tm = np.asarray(w1, np.float32).T.astype(ml_dtypes.bfloat16)
    w2Tm = np.asarray(w2, np.float32).T.astype(ml_dtypes.bfloat16)
    w1tt = np.ascontiguousarray(
        w1Tm.reshape(16, 128, 64, 128).transpose(2, 1, 0, 3)).reshape(64, 128, 2048)
    base = {
        "xr": np.ascontiguousarray(xr),
        "w1tt": w1tt,
        "rwT": np.ascontiguousarray(np.asarray(router_w, np.float32).T),
        "w2T": np.ascontiguousarray(w2Tm),
        "b1t": np.ascontiguousarray(np.asarray(b1, np.float32).reshape(FT, 128).T),
        "b2r": np.asarray(b2, np.float32).reshape(1, H),
    }
    xtmf = np.ascontiguousarray(
        xT.reshape(16, 128, 16, 128).transpose(2, 1, 0, 3)).reshape(16, 128, 2048)
    ins = []
    for c in range(NCORES):
        m = dict(base)
        m["xtm"] = np.ascontiguousarray(xtmf[2 * c:2 * c + 2])
        m["cid"] = np.full((1, 1), float(c), np.float32)
        ins.append(m)
    return ins


def _get_nc():
    if "nc" not in _CACHE:
        apply()  # tile drain patch
        nc = bass.Bass(num_devices=NCORES)
        build_moe(nc)
        split_multi_waits(nc)
        _CACHE["nc"] = nc
    return _CACHE["nc"]


def kernel(hidden_states, router_w, w1, b1, w2, b2):
    from concourse.bass_utils import run_bass_kernel_spmd

    orig_shape = np.asarray(hidden_states).shape
    nc = _get_nc()
    ins = _prep_inputs(hidden_states, router_w, w1, b1, w2, b2)
    res = run_bass_kernel_spmd(nc, ins, core_ids=list(range(NCORES)))
    full = np.concatenate([res.results[c]["out"] for c in range(NCORES)], axis=0)
    return full.reshape(orig_shape).astype(np.float32)



# revision 15
# speedup vs baseline: 2.3837x; 2.3837x over previous
"""Trainium2 8-core MoE layer kernel (collapsed shared-expert FFN, Bass/Tile).

The reference MoE applies the SAME w1/b1/w2/b2 to every expert's slice of
the dispatch buffer, so the whole layer collapses algebraically to

    out[t] = s(t) * (gelu(x[t] @ w1.T + b1) @ w2.T + b2)
    s(t)   = p0(t)*keep0(t) + p1(t)*keep1(t)

where keep_k(t) is the capacity-survival flag from the slot-major
cumulative-count over the global top-2 assignment sequence.  Only s(t)
needs global routing info; the FFN itself is a dense per-token FFN.

Sharding: token-parallel, 256 tokens per core.  Each core runs the
fp32 router on its own tokens, AllGathers the 4KB of routing decisions,
recomputes the global cumsum locally (replicated, deterministic), and
runs the dense FFN on its 256 tokens with w1/w2 streamed from HBM in
bf16.  The routing/scan chain runs on vector/scalar/gpsimd engines
concurrently with the F1 matmul stream; s(t) is only consumed by the
final epilogue scaling.
"""
import numpy as np
import ml_dtypes

import concourse.bass as bass
import concourse.mybir as mybir
import concourse.tile as tile

_PATCH_DOC = """Patch TileContext._drain_and_barrier: the stock version stuffs every
outstanding semaphore wait onto one SP Drain instruction; the installed
walrus rejects >1 sync wait per non-EventSemaphore instruction
("Too many sync wait commands"). Split the waits across a chain of SP
nops, then drain/barrier as before."""
import concourse.tile as tile_mod
from concourse.vector_clock import ScopedClock


def _patched_drain_and_barrier(self, tick_clock, wait_clock):
    nc = self.nc
    carrier = nc.sync.nop(nofuse=True, hint="drain_wait_carrier")
    wait_clock.add_sem_waits(
        carrier.ins, ScopedClock({None: tick_clock.global_clock})
    )
    waits = list(carrier.ins.sync_info.on_wait)
    if len(waits) > 1:
        carrier.ins.sync_info.on_wait = waits[:1]
        import bass_rust as _br
        for w in waits[1:]:
            extra = nc.sync.nop(nofuse=True, hint="drain_wait_carrier")
            extra.ins.sync_info = _br.SyncInfo(on_wait=[w], on_update=[])

    nc.sync.drain()
    nc.all_engine_barrier()
    assert self.sems is not None
    popped = nc._tile_sem_poison_stack.pop()
    assert popped is self._sem_poison
    nc.clear_and_free_semaphores(list(self.sems.allocated().values()))
    nc.all_engine_barrier()


def apply():
    tile_mod.TileContext._drain_and_barrier = _patched_drain_and_barrier


import concourse.mybir as mybir
import bass_rust as _br


def split_multi_waits(nc):
    """Walrus in this container accepts at most ONE sync wait per
    instruction. Hoist extra waits onto same-engine NoOps inserted
    immediately before the offending instruction."""
    ctr = 0
    for f in nc.m.functions:
        for b in f.blocks:
            insts = b.instructions
            need = any(
                inst.sync_info is not None and len(inst.sync_info.on_wait) > 1
                for inst in insts
            )
            if not need:
                continue
            out = []
            for inst in insts:
                si = inst.sync_info
                if si is not None and len(si.on_wait) > 1:
                    waits = list(si.on_wait)
                    for w in waits[:-1]:
                        nop = mybir.InstNoOp(name=f"I-wsplit-{ctr}", ins=[], outs=[])
                        ctr += 1
                        nop.engine = inst.engine
                        nop.sync_info = _br.SyncInfo(on_wait=[w], on_update=[])
                        out.append(nop)
                    si.on_wait = waits[-1:]
                out.append(inst)
            b.instructions = out
    return ctr


E, TOPK, CAP, H, F, N, NCORES = 8, 2, 512, 2048, 8192, 2048, 8
TT = N // 128                # 16 token tiles (global)
HT = H // 128                # 16 hidden tiles
FT = F // 128                # 64 ffn tiles
TOKC = N // NCORES           # 256 tokens per core
MT = TOKC // 128             # 2 local token tiles

f32 = mybir.dt.float32
f16 = mybir.dt.float16
bf16 = mybir.dt.bfloat16
i32 = mybir.dt.int32
AOP = mybir.AluOpType
AFT = mybir.ActivationFunctionType
AX = mybir.AxisListType


def build_moe(nc: bass.Bass):
    xtm = nc.dram_tensor("xtm", [MT, 128, H], f32, kind="ExternalInput")
    xtc = nc.dram_tensor("xtc", [HT, 128, TOKC], bf16, kind="ExternalInput")
    rwT = nc.dram_tensor("rwT", [H, E], f32, kind="ExternalInput")
    w1T = nc.dram_tensor("w1tt", [FT, 128, HT * 128], bf16, kind="ExternalInput")
    w2T = nc.dram_tensor("w2T", [F, H], bf16, kind="ExternalInput")
    b1t = nc.dram_tensor("b1t", [128, FT], f32, kind="ExternalInput")
    b2r = nc.dram_tensor("b2r", [1, H], f32, kind="ExternalInput")
    cid = nc.dram_tensor("cid", [1, 1], f32, kind="ExternalInput")
    out = nc.dram_tensor("out", [TOKC, H], f32, kind="ExternalOutput")

    rloc = nc.dram_tensor("rloc", [2, TOKC], f32)
    rall = nc.dram_tensor("rall", [2 * NCORES, TOKC], f32, addr_space="Shared")
    posd = nc.dram_tensor("posd", [1, 2 * N], f32)
    posq = nc.dram_tensor("posq", [2 * MT, 128], f32)

    with tile.TileContext(nc, num_cores=NCORES) as tc:
        with tc.tile_pool(name="persist", bufs=1) as persist:
            _body(nc, tc, persist, xtm, xtc, rwT, w1T, w2T, b1t, b2r, cid,
                  out, rloc, rall, posd, posq)
    return nc


def _body(nc, tc, persist, xtm, xtc, rwT, w1T, w2T, b1t, b2r, cid, out,
          rloc, rall, posd, posq):
    # ---- persistent tiles ----
    xts = persist.tile([128, HT * TOKC], bf16, tag="xts")      # xT own tokens
    b2b = persist.tile([128, H], f32, tag="b2b")
    b2s = [persist.tile([128, H], f32, tag=f"b2s_{m}", name=f"b2s_{m}")
           for m in range(MT)]
    cidb4 = persist.tile([2 * MT, 1], f32, tag="cidb4")
    b1sb = persist.tile([128, FT], f32, tag="b1sb")
    rws = persist.tile([128, HT * E], f32, tag="rws")
    sm = [persist.tile([128, 1], f32, tag=f"sm_{m}", name=f"sm_{m}")
          for m in range(MT)]
    prb = [persist.tile([128, 2], f32, tag=f"prb_{m}", name=f"prb_{m}")
           for m in range(MT)]
    posti = persist.tile([128, 2 * MT], f32, tag="posti")

    # pools in LIFO lifetime order (longest-lived entered first)
    g_cm = tc.tile_pool(name="g", bufs=1)
    g_pool = g_cm.__enter__()
    w2_cm = tc.tile_pool(name="w2s", bufs=6)
    w2_pool = w2_cm.__enter__()
    scan_cm = tc.tile_pool(name="scan", bufs=1)
    scan_pool = scan_cm.__enter__()
    w1_cm = tc.tile_pool(name="w1s", bufs=8)
    w1_pool = w1_cm.__enter__()
    f1ps_cm = tc.tile_pool(name="f1ps", bufs=4, space="PSUM")
    f1ps_pool = f1ps_cm.__enter__()

    # initial small loads: gpsimd handles router-critical, scalar the rest
    nc.gpsimd.dma_start(out=cidb4[:],
                        in_=cid[0:1, :].partition_broadcast(2 * MT).opt())
    nc.gpsimd.dma_start(out=rws[:].rearrange("p (c e) -> p c e", e=E),
                        in_=rwT[:, :].rearrange("(c p) e -> p c e", p=128))
    nc.scalar.dma_start(out=xts[:].rearrange("p (c t) -> p c t", t=TOKC),
                        in_=xtc[:, :, :].rearrange("c p t -> p c t"))
    nc.scalar.dma_start(out=b1sb[:], in_=b1t[:, :])

    w1t = {}

    def load_w1(ft):
        t = w1_pool.tile([128, HT * 128], bf16, tag="w1t")
        w1t[ft] = t
        eng = nc.sync if (ft % 2 == 0) else nc.scalar
        eng.dma_start(out=t[:], in_=w1T[ft, :, :])

    for ft in range(6):
        load_w1(ft)
    nc.scalar.dma_start(out=b2b[:], in_=b2r[0:1, :].partition_broadcast(128).opt())

    g = [g_pool.tile([128, TOKC], bf16, tag=f"g_{ft}", name=f"g_{ft}")
         for ft in range(FT)]

    def f1_block(ft):
        if ft + 6 < FT:
            load_w1(ft + 6)
        ps = f1ps_pool.tile([128, TOKC], f32, tag="f1ps")
        for hc in range(HT):
            nc.tensor.matmul(
                out=ps[:], lhsT=w1t[ft][:, hc * 128:(hc + 1) * 128],
                rhs=xts[:, hc * TOKC:(hc + 1) * TOKC],
                start=(hc == 0), stop=(hc == HT - 1))
        nc.scalar.activation(out=g[ft][:], in_=ps[:], func=AFT.Gelu,
                             bias=b1sb[:, ft:ft + 1], scale=1.0)

    # F1 head: two tiles before the router matmuls so the PE queue never
    # waits on the (slightly later) xtm DMA.
    f1_block(0)
    f1_block(1)

    # ============ Router (own 256 tokens, fp32) + AllGather ============
    with (tc.tile_pool(name="r_x", bufs=2) as r_x,
          tc.tile_pool(name="r_ps", bufs=2, space="PSUM") as r_ps,
          tc.tile_pool(name="r_sb", bufs=2) as r_sb):
        for tt2 in range(MT):
            xt_t = r_x.tile([128, H], f32, tag="xt_t")
            nc.gpsimd.dma_start(out=xt_t[:], in_=xtm[tt2, :, :])
            ps = r_ps.tile([128, E], f32, tag="r_ps")
            for hc in range(HT):
                nc.tensor.matmul(
                    out=ps[:], lhsT=xt_t[:, hc * 128:(hc + 1) * 128],
                    rhs=rws[:, hc * E:(hc + 1) * E],
                    start=(hc == 0), stop=(hc == HT - 1))
            lsb = r_sb.tile([128, E], f32, tag="lsb")
            nc.vector.tensor_copy(out=lsb[:], in_=ps[:])
            mx = r_sb.tile([128, 1], f32, tag="mx")
            nc.vector.tensor_reduce(out=mx[:], in_=lsb[:], op=AOP.max, axis=AX.X)
            nm = r_sb.tile([128, 1], f32, tag="nm")
            nc.vector.tensor_scalar_mul(nm[:], mx[:], -1.0)
            ex = r_sb.tile([128, E], f32, tag="ex")
            ssum = r_sb.tile([128, 1], f32, tag="ssum")
            nc.scalar.activation(out=ex[:], in_=lsb[:], func=AFT.Exp,
                                 bias=nm[:], scale=1.0, accum_out=ssum[:])
            rcp = r_sb.tile([128, 1], f32, tag="rcp")
            nc.vector.reciprocal(out=rcp[:], in_=ssum[:])
            pr = r_sb.tile([128, E], f32, tag="pr")
            nc.vector.tensor_scalar_mul(pr[:], ex[:], rcp[:])
            mx8 = r_sb.tile([128, 8], f32, tag="mx8")
            ix8 = r_sb.tile([128, 8], mybir.dt.uint32, tag="ix8")
            nc.vector.max_with_indices(out_max=mx8[:], out_indices=ix8[:],
                                       in_=pr[:])
            ev = r_sb.tile([128, 2], f32, tag="ev")
            nc.vector.tensor_copy(out=ev[:, 0:1], in_=ix8[:, 0:1])
            nc.vector.tensor_copy(out=ev[:, 1:2], in_=ix8[:, 1:2])
            nc.vector.tensor_copy(out=prb[tt2][:], in_=mx8[:, 0:2])
            nc.gpsimd.dma_start(
                out=rloc[:, tt2 * 128:(tt2 + 1) * 128].rearrange("a t -> t a"),
                in_=ev[:])
        nc.gpsimd.collective_compute(
            "AllGather", AOP.bypass,
            replica_groups=[list(range(NCORES))],
            ins=[rloc[:, :].opt()],
            outs=[rall[:, :].opt()])

    # w2 prefetch for the F1->F2 boundary (gpsimd queue, right after AG)
    w2t = {}

    def load_w2(fc, eng=None):
        t = w2_pool.tile([128, H], bf16, tag="w2t")
        w2t[fc] = t
        if eng is None:
            eng = nc.sync if (fc % 2 == 0) else nc.scalar
        eng.dma_start(out=t[:], in_=w2T[fc * 128:(fc + 1) * 128, :])

    load_w2(0, eng=nc.gpsimd)
    load_w2(1, eng=nc.gpsimd)

    # ============ Scan: global slot-major position per assignment ======
    # (vector engine; runs concurrently with F1 matmuls)
    iop8 = scan_pool.tile([E, 1], i32, tag="iop8")
    iop8f = scan_pool.tile([E, 1], f32, tag="iop8f")
    nc.gpsimd.iota(iop8[:], pattern=[[0, 1]], base=0, channel_multiplier=1)
    nc.vector.tensor_copy(out=iop8f[:], in_=iop8[:])
    e0b = scan_pool.tile([E, N], f16, tag="e0b")
    e1b = scan_pool.tile([E, N], f16, tag="e1b")
    rallv = rall[:, :].rearrange("(c a) t -> a c t", a=2)
    nc.gpsimd.dma_start(
        out=e0b[:].rearrange("p (c t) -> p c t", t=TOKC),
        in_=rallv[0:1, :, :].partition_broadcast(E).opt())
    nc.gpsimd.dma_start(
        out=e1b[:].rearrange("p (c t) -> p c t", t=TOKC),
        in_=rallv[1:2, :, :].partition_broadcast(E).opt())
    ohcat = scan_pool.tile([E, 2 * N], f16, tag="ohcat")
    nc.vector.tensor_scalar(out=ohcat[:, :N], in0=e0b[:], scalar1=iop8f[:],
                            scalar2=None, op0=AOP.is_equal)
    nc.vector.tensor_scalar(out=ohcat[:, N:], in0=e1b[:], scalar1=iop8f[:],
                            scalar2=None, op0=AOP.is_equal)
    ones2n = scan_pool.tile([E, 2 * N], f16, tag="ones2n")
    nc.vector.memset(ones2n[:], 1.0)
    cum = scan_pool.tile([E, 2 * N], f16, tag="cum")
    nc.vector.tensor_tensor_scan(out=cum[:], data0=ones2n[:], data1=ohcat[:],
                                 initial=0.0, op0=AOP.mult, op1=AOP.add)
    ohcum = scan_pool.tile([E, 2 * N], f16, tag="ohcum")
    nc.vector.tensor_tensor(out=ohcum[:], in0=ohcat[:], in1=cum[:],
                            op=AOP.mult)
    ones8 = scan_pool.tile([E, 1], f16, tag="ones8")
    nc.vector.memset(ones8[:], 1.0)
    posrow = scan_pool.tile([1, 2 * N], f32, tag="posrow")

    # ============ F1 body (tiles 2..31) ============
    for ft in range(2, 32):
        f1_block(ft)

    # pos extraction matmuls dropped into the middle of the F1 stream:
    # operands are ready by now, keeps the tail free.
    with tc.tile_pool(name="csps", bufs=2, space="PSUM") as csps:
        for ch in range(2 * N // 512):
            pps = csps.tile([1, 512], f32, tag="pps")
            nc.tensor.matmul(out=pps[:], lhsT=ones8[:],
                             rhs=ohcum[:, ch * 512:(ch + 1) * 512],
                             start=True, stop=True)
            nc.vector.tensor_scalar_add(
                posrow[:, ch * 512:(ch + 1) * 512], pps[:], -1.0)
    nc.gpsimd.dma_start(out=posd[:, :], in_=posrow[:])

    for ft in range(32, FT):
        f1_block(ft)

    # ============ own-token keep/s: gather pos rows {2c+m, 16+2c+m} ======
    f1ps_cm.__exit__(None, None, None)
    w1_cm.__exit__(None, None, None)

    with tc.tile_pool(name="imath", bufs=1) as im:
        iop4 = im.tile([2 * MT, 1], i32, tag="iop4")
        iop4f = im.tile([2 * MT, 1], f32, tag="iop4f")
        nc.gpsimd.iota(iop4[:], pattern=[[0, 1]], base=0, channel_multiplier=1)
        nc.vector.tensor_copy(out=iop4f[:], in_=iop4[:])
        ge2 = im.tile([2 * MT, 1], f32, tag="ge2")
        nc.vector.tensor_scalar(out=ge2[:], in0=iop4f[:], scalar1=float(MT),
                                scalar2=None, op0=AOP.is_ge)
        idxa = im.tile([2 * MT, 1], f32, tag="idxa")
        nc.vector.scalar_tensor_tensor(out=idxa[:], in0=ge2[:],
                                       scalar=float(TT - MT), in1=iop4f[:],
                                       op0=AOP.mult, op1=AOP.add)
        idx4f = im.tile([2 * MT, 1], f32, tag="idx4f")
        nc.vector.scalar_tensor_tensor(out=idx4f[:], in0=cidb4[:],
                                       scalar=float(MT), in1=idxa[:],
                                       op0=AOP.mult, op1=AOP.add)
        idx4i = im.tile([2 * MT, 1], i32, tag="idx4i")
        nc.vector.tensor_copy(out=idx4i[:], in_=idx4f[:])
        pos4 = im.tile([2 * MT, 128], f32, tag="pos4")
        nc.gpsimd.indirect_dma_start(
            out=pos4[:], out_offset=None,
            in_=posd[0:1, :].rearrange("a (r p) -> (a r) p", p=128),
            in_offset=bass.IndirectOffsetOnAxis(ap=idx4i[:, :1], axis=0))
        nc.gpsimd.dma_start(out=posq[:, :], in_=pos4[:])
        nc.gpsimd.dma_start(out=posti[:],
                            in_=posq[:, :].rearrange("r p -> p r"))
        keep = im.tile([128, 2 * MT], f32, tag="keep")
        nc.vector.tensor_scalar(out=keep[:], in0=posti[:], scalar1=float(CAP),
                                scalar2=None, op0=AOP.is_lt)
        for m in range(MT):
            sa = im.tile([128, 1], f32, tag="sa")
            nc.vector.tensor_tensor(out=sa[:], in0=prb[m][:, 0:1],
                                    in1=keep[:, m:m + 1], op=AOP.mult)
            sb = im.tile([128, 1], f32, tag="sb")
            nc.vector.tensor_tensor(out=sb[:], in0=prb[m][:, 1:2],
                                    in1=keep[:, MT + m:MT + m + 1],
                                    op=AOP.mult)
            nc.vector.tensor_tensor(out=sm[m][:], in0=sa[:], in1=sb[:],
                                    op=AOP.add)
        for m in range(MT):
            nc.vector.tensor_scalar_mul(b2s[m][:], b2b[:], sm[m][:, 0:1])

    scan_cm.__exit__(None, None, None)

    # ============ F2: y = g @ w2.T, scaled epilogue ============
    for fc in range(2, 6):
        load_w2(fc)

    with (tc.tile_pool(name="f2ps", bufs=1, space="PSUM") as f2ps,
          tc.tile_pool(name="f2o", bufs=4) as f2o):
        psq = [[f2ps.tile([128, 512], f32, tag=f"f2ps_{m}_{hq}",
                          name=f"f2ps_{m}_{hq}")
                for hq in range(4)] for m in range(MT)]
        for fc in range(FT):
            if fc + 4 < FT:
                load_w2(fc + 4)
            for m in range(MT):
                for hq in range(4):
                    nc.tensor.matmul(
                        out=psq[m][hq][:],
                        lhsT=g[fc][:, m * 128:(m + 1) * 128],
                        rhs=w2t[fc][:, hq * 512:(hq + 1) * 512],
                        start=(fc == 0), stop=(fc == FT - 1))
        for m in range(MT):
            for hq in range(4):
                o_t = f2o.tile([128, 512], f32, tag="o_t")
                nc.vector.scalar_tensor_tensor(
                    out=o_t[:], in0=psq[m][hq][:], scalar=sm[m][:, 0:1],
                    in1=b2s[m][:, hq * 512:(hq + 1) * 512],
                    op0=AOP.mult, op1=AOP.add)
                eng = nc.gpsimd if (hq % 2 == 0) else nc.sync
                eng.dma_start(
                    out=out[m * 128:(m + 1) * 128, hq * 512:(hq + 1) * 512],
                    in_=o_t[:])

    w2_cm.__exit__(None, None, None)
    g_cm.__exit__(None, None, None)


# ======================== host-side glue ========================

_CACHE = {}


def _prep_inputs(hidden_states, router_w, w1, b1, w2, b2):
    x = np.asarray(hidden_states, np.float32).reshape(-1, H)
    xT = np.ascontiguousarray(x.T)                       # [H, N] fp32
    w1Tm = np.asarray(w1, np.float32).T.astype(ml_dtypes.bfloat16)
    w2Tm = np.asarray(w2, np.float32).T.astype(ml_dtypes.bfloat16)
    w1tt = np.ascontiguousarray(
        w1Tm.reshape(HT, 128, FT, 128).transpose(2, 1, 0, 3)).reshape(
            FT, 128, H)
    base = {
        "w1tt": w1tt,
        "rwT": np.ascontiguousarray(np.asarray(router_w, np.float32).T),
        "w2T": np.ascontiguousarray(w2Tm),
        "b1t": np.ascontiguousarray(np.asarray(b1, np.float32).reshape(FT, 128).T),
        "b2r": np.asarray(b2, np.float32).reshape(1, H),
    }
    # router layout: [t_outer, h_inner, h_outer*t_inner] fp32
    xtmf = np.ascontiguousarray(
        xT.reshape(HT, 128, TT, 128).transpose(2, 1, 0, 3)).reshape(TT, 128, H)
    # FFN rhs layout: xT in [h_outer, h_inner, token] bf16, per-core slice
    xTb = xT.astype(ml_dtypes.bfloat16)
    ins = []
    for c in range(NCORES):
        m = dict(base)
        m["xtm"] = np.ascontiguousarray(xtmf[MT * c:MT * (c + 1)])
        m["xtc"] = np.ascontiguousarray(
            xTb[:, c * TOKC:(c + 1) * TOKC].reshape(HT, 128, TOKC))
        m["cid"] = np.full((1, 1), float(c), np.float32)
        ins.append(m)
    return ins


def _get_nc():
    if "nc" not in _CACHE:
        apply()  # tile drain patch
        nc = bass.Bass(num_devices=NCORES)
        build_moe(nc)
        split_multi_waits(nc)
        _CACHE["nc"] = nc
    return _CACHE["nc"]


def kernel(hidden_states, router_w, w1, b1, w2, b2):
    from concourse.bass_utils import run_bass_kernel_spmd

    orig_shape = np.asarray(hidden_states).shape
    nc = _get_nc()
    ins = _prep_inputs(hidden_states, router_w, w1, b1, w2, b2)
    res = run_bass_kernel_spmd(nc, ins, core_ids=list(range(NCORES)))
    full = np.concatenate([res.results[c]["out"] for c in range(NCORES)], axis=0)
    return full.reshape(orig_shape).astype(np.float32)


# revision 16
# speedup vs baseline: 2.4848x; 1.0424x over previous
"""Trainium2 8-core MoE layer kernel (collapsed shared-expert FFN, Bass/Tile).

The reference MoE applies the SAME w1/b1/w2/b2 to every expert's slice of
the dispatch buffer, so the whole layer collapses algebraically to

    out[t] = s(t) * (gelu(x[t] @ w1.T + b1) @ w2.T + b2)
    s(t)   = p0(t)*keep0(t) + p1(t)*keep1(t)

where keep_k(t) is the capacity-survival flag from the slot-major
cumulative-count over the global top-2 assignment sequence.  Only s(t)
needs global routing info; the FFN itself is a dense per-token FFN.

Sharding: token-parallel, 256 tokens per core.  Each core runs the
fp32 router on its own tokens, AllGathers the 4KB of routing decisions,
recomputes the global cumsum locally (replicated, deterministic), and
runs the dense FFN on its 256 tokens with w1/w2 streamed from HBM in
bf16.  The routing/scan chain runs on vector/scalar/gpsimd engines
concurrently with the F1 matmul stream; s(t) is only consumed by the
final epilogue scaling.
"""
import numpy as np
import ml_dtypes

import concourse.bass as bass
import concourse.mybir as mybir
import concourse.tile as tile

_PATCH_DOC = """Patch TileContext._drain_and_barrier: the stock version stuffs every
outstanding semaphore wait onto one SP Drain instruction; the installed
walrus rejects >1 sync wait per non-EventSemaphore instruction
("Too many sync wait commands"). Split the waits across a chain of SP
nops, then drain/barrier as before."""
import concourse.tile as tile_mod
from concourse.vector_clock import ScopedClock


def _patched_drain_and_barrier(self, tick_clock, wait_clock):
    nc = self.nc
    carrier = nc.sync.nop(nofuse=True, hint="drain_wait_carrier")
    wait_clock.add_sem_waits(
        carrier.ins, ScopedClock({None: tick_clock.global_clock})
    )
    waits = list(carrier.ins.sync_info.on_wait)
    if len(waits) > 1:
        carrier.ins.sync_info.on_wait = waits[:1]
        import bass_rust as _br
        for w in waits[1:]:
            extra = nc.sync.nop(nofuse=True, hint="drain_wait_carrier")
            extra.ins.sync_info = _br.SyncInfo(on_wait=[w], on_update=[])

    nc.sync.drain()
    nc.all_engine_barrier()
    assert self.sems is not None
    popped = nc._tile_sem_poison_stack.pop()
    assert popped is self._sem_poison
    nc.clear_and_free_semaphores(list(self.sems.allocated().values()))
    nc.all_engine_barrier()


def apply():
    tile_mod.TileContext._drain_and_barrier = _patched_drain_and_barrier


import concourse.mybir as mybir
import bass_rust as _br


def split_multi_waits(nc):
    """Walrus in this container accepts at most ONE sync wait per
    instruction. Hoist extra waits onto same-engine NoOps inserted
    immediately before the offending instruction."""
    ctr = 0
    for f in nc.m.functions:
        for b in f.blocks:
            insts = b.instructions
            need = any(
                inst.sync_info is not None and len(inst.sync_info.on_wait) > 1
                for inst in insts
            )
            if not need:
                continue
            out = []
            for inst in insts:
                si = inst.sync_info
                if si is not None and len(si.on_wait) > 1:
                    waits = list(si.on_wait)
                    for w in waits[:-1]:
                        nop = mybir.InstNoOp(name=f"I-wsplit-{ctr}", ins=[], outs=[])
                        ctr += 1
                        nop.engine = inst.engine
                        nop.sync_info = _br.SyncInfo(on_wait=[w], on_update=[])
                        out.append(nop)
                    si.on_wait = waits[-1:]
                out.append(inst)
            b.instructions = out
    return ctr


E, TOPK, CAP, H, F, N, NCORES = 8, 2, 512, 2048, 8192, 2048, 8
TT = N // 128                # 16 token tiles (global)
HT = H // 128                # 16 hidden tiles
FT = F // 128                # 64 ffn tiles
TOKC = N // NCORES           # 256 tokens per core
MT = TOKC // 128             # 2 local token tiles

f32 = mybir.dt.float32
f16 = mybir.dt.float16
bf16 = mybir.dt.bfloat16
i32 = mybir.dt.int32
AOP = mybir.AluOpType
AFT = mybir.ActivationFunctionType
AX = mybir.AxisListType


def build_moe(nc: bass.Bass):
    xtm = nc.dram_tensor("xtm", [MT, 128, H], f32, kind="ExternalInput")
    xtc = nc.dram_tensor("xtc", [128, HT * TOKC], bf16, kind="ExternalInput")
    rwT = nc.dram_tensor("rwT", [H, E], f32, kind="ExternalInput")
    w1T = nc.dram_tensor("w1tt", [FT, 128, HT * 128], bf16, kind="ExternalInput")
    w2T = nc.dram_tensor("w2T", [F, H], bf16, kind="ExternalInput")
    b1t = nc.dram_tensor("b1t", [128, FT], f32, kind="ExternalInput")
    b2r = nc.dram_tensor("b2r", [1, H], f32, kind="ExternalInput")
    cid = nc.dram_tensor("cid", [1, 1], f32, kind="ExternalInput")
    out = nc.dram_tensor("out", [TOKC, H], f32, kind="ExternalOutput")

    rloc = nc.dram_tensor("rloc", [2, TOKC], f32)
    rall = nc.dram_tensor("rall", [2 * NCORES, TOKC], f32, addr_space="Shared")
    posd = nc.dram_tensor("posd", [1, 2 * N], f32)
    posq = nc.dram_tensor("posq", [2 * MT, 128], f32)

    with tile.TileContext(nc, num_cores=NCORES) as tc:
        with tc.tile_pool(name="persist", bufs=1) as persist:
            _body(nc, tc, persist, xtm, xtc, rwT, w1T, w2T, b1t, b2r, cid,
                  out, rloc, rall, posd, posq)
    return nc


def _body(nc, tc, persist, xtm, xtc, rwT, w1T, w2T, b1t, b2r, cid, out,
          rloc, rall, posd, posq):
    # ---- persistent tiles ----
    xts = persist.tile([128, HT * TOKC], bf16, tag="xts")      # xT own tokens
    b2b = persist.tile([128, H], f32, tag="b2b")
    b2s = [persist.tile([128, H], f32, tag=f"b2s_{m}", name=f"b2s_{m}")
           for m in range(MT)]
    cidb4 = persist.tile([2 * MT, 1], f32, tag="cidb4")
    b1sb = persist.tile([128, FT], f32, tag="b1sb")
    rws = persist.tile([128, HT * E], f32, tag="rws")
    sm = [persist.tile([128, 1], f32, tag=f"sm_{m}", name=f"sm_{m}")
          for m in range(MT)]
    prb = [persist.tile([128, 2], f32, tag=f"prb_{m}", name=f"prb_{m}")
           for m in range(MT)]
    posti = persist.tile([128, 2 * MT], f32, tag="posti")

    # pools in LIFO lifetime order (longest-lived entered first)
    g_cm = tc.tile_pool(name="g", bufs=1)
    g_pool = g_cm.__enter__()
    w2_cm = tc.tile_pool(name="w2s", bufs=6)
    w2_pool = w2_cm.__enter__()
    scan_cm = tc.tile_pool(name="scan", bufs=1)
    scan_pool = scan_cm.__enter__()
    w1_cm = tc.tile_pool(name="w1s", bufs=8)
    w1_pool = w1_cm.__enter__()
    f1ps_cm = tc.tile_pool(name="f1ps", bufs=6, space="PSUM")
    f1ps_pool = f1ps_cm.__enter__()

    # initial small loads: gpsimd handles router-critical, scalar the rest
    nc.gpsimd.dma_start(out=cidb4[:],
                        in_=cid[0:1, :].partition_broadcast(2 * MT).opt())
    nc.gpsimd.dma_start(out=rws[:].rearrange("p (c e) -> p c e", e=E),
                        in_=rwT[:, :].rearrange("(c p) e -> p c e", p=128))
    nc.scalar.dma_start(out=xts[:], in_=xtc[:, :])
    nc.scalar.dma_start(out=b1sb[:], in_=b1t[:, :])

    w1t = {}

    def load_w1(ft):
        t = w1_pool.tile([128, HT * 128], bf16, tag="w1t")
        w1t[ft] = t
        eng = nc.sync if (ft % 2 == 0) else nc.scalar
        eng.dma_start(out=t[:], in_=w1T[ft, :, :])

    for ft in range(6):
        load_w1(ft)
    nc.scalar.dma_start(out=b2b[:], in_=b2r[0:1, :].partition_broadcast(128).opt())

    g = [g_pool.tile([128, TOKC], bf16, tag=f"g_{ft}", name=f"g_{ft}")
         for ft in range(FT)]

    def f1_block(ft):
        if ft + 6 < FT:
            load_w1(ft + 6)
        ps = f1ps_pool.tile([128, TOKC], f32, tag="f1ps")
        for hc in range(HT):
            nc.tensor.matmul(
                out=ps[:], lhsT=w1t[ft][:, hc * 128:(hc + 1) * 128],
                rhs=xts[:, hc * TOKC:(hc + 1) * TOKC],
                start=(hc == 0), stop=(hc == HT - 1))
        nc.scalar.activation(out=g[ft][:], in_=ps[:], func=AFT.Gelu,
                             bias=b1sb[:, ft:ft + 1], scale=1.0)

    # F1 head: two tiles before the router matmuls so the PE queue never
    # waits on the (slightly later) xtm DMA.
    f1_block(0)
    f1_block(1)

    # ============ Router (own 256 tokens, fp32) + AllGather ============
    with (tc.tile_pool(name="r_x", bufs=2) as r_x,
          tc.tile_pool(name="r_ps", bufs=2, space="PSUM") as r_ps,
          tc.tile_pool(name="r_sb", bufs=2) as r_sb):
        for tt2 in range(MT):
            xt_t = r_x.tile([128, H], f32, tag="xt_t")
            nc.gpsimd.dma_start(out=xt_t[:], in_=xtm[tt2, :, :])
            ps = r_ps.tile([128, E], f32, tag="r_ps")
            for hc in range(HT):
                nc.tensor.matmul(
                    out=ps[:], lhsT=xt_t[:, hc * 128:(hc + 1) * 128],
                    rhs=rws[:, hc * E:(hc + 1) * E],
                    start=(hc == 0), stop=(hc == HT - 1))
            lsb = r_sb.tile([128, E], f32, tag="lsb")
            nc.vector.tensor_copy(out=lsb[:], in_=ps[:])
            mx = r_sb.tile([128, 1], f32, tag="mx")
            nc.vector.tensor_reduce(out=mx[:], in_=lsb[:], op=AOP.max, axis=AX.X)
            nm = r_sb.tile([128, 1], f32, tag="nm")
            nc.vector.tensor_scalar_mul(nm[:], mx[:], -1.0)
            ex = r_sb.tile([128, E], f32, tag="ex")
            ssum = r_sb.tile([128, 1], f32, tag="ssum")
            nc.scalar.activation(out=ex[:], in_=lsb[:], func=AFT.Exp,
                                 bias=nm[:], scale=1.0, accum_out=ssum[:])
            rcp = r_sb.tile([128, 1], f32, tag="rcp")
            nc.vector.reciprocal(out=rcp[:], in_=ssum[:])
            pr = r_sb.tile([128, E], f32, tag="pr")
            nc.vector.tensor_scalar_mul(pr[:], ex[:], rcp[:])
            mx8 = r_sb.tile([128, 8], f32, tag="mx8")
            ix8 = r_sb.tile([128, 8], mybir.dt.uint32, tag="ix8")
            nc.vector.max_with_indices(out_max=mx8[:], out_indices=ix8[:],
                                       in_=pr[:])
            ev = r_sb.tile([128, 2], f32, tag="ev")
            nc.vector.tensor_copy(out=ev[:, 0:1], in_=ix8[:, 0:1])
            nc.vector.tensor_copy(out=ev[:, 1:2], in_=ix8[:, 1:2])
            nc.vector.tensor_copy(out=prb[tt2][:], in_=mx8[:, 0:2])
            nc.gpsimd.dma_start(
                out=rloc[:, tt2 * 128:(tt2 + 1) * 128].rearrange("a t -> t a"),
                in_=ev[:])
        nc.gpsimd.collective_compute(
            "AllGather", AOP.bypass,
            replica_groups=[list(range(NCORES))],
            ins=[rloc[:, :].opt()],
            outs=[rall[:, :].opt()])

    # w2 prefetch for the F1->F2 boundary (gpsimd queue, right after AG)
    w2t = {}

    def load_w2(fc, eng=None):
        t = w2_pool.tile([128, H], bf16, tag="w2t")
        w2t[fc] = t
        if eng is None:
            eng = nc.sync if (fc % 2 == 0) else nc.scalar
        eng.dma_start(out=t[:], in_=w2T[fc * 128:(fc + 1) * 128, :])

    load_w2(0, eng=nc.gpsimd)
    load_w2(1, eng=nc.gpsimd)

    # ============ Scan: global slot-major position per assignment ======
    # (vector engine; runs concurrently with F1 matmuls)
    iop8 = scan_pool.tile([E, 1], i32, tag="iop8")
    iop8f = scan_pool.tile([E, 1], f32, tag="iop8f")
    nc.gpsimd.iota(iop8[:], pattern=[[0, 1]], base=0, channel_multiplier=1)
    nc.vector.tensor_copy(out=iop8f[:], in_=iop8[:])
    e0b = scan_pool.tile([E, N], f16, tag="e0b")
    e1b = scan_pool.tile([E, N], f16, tag="e1b")
    rallv = rall[:, :].rearrange("(c a) t -> a c t", a=2)
    nc.gpsimd.dma_start(
        out=e0b[:].rearrange("p (c t) -> p c t", t=TOKC),
        in_=rallv[0:1, :, :].partition_broadcast(E).opt())
    nc.gpsimd.dma_start(
        out=e1b[:].rearrange("p (c t) -> p c t", t=TOKC),
        in_=rallv[1:2, :, :].partition_broadcast(E).opt())
    ohcat = scan_pool.tile([E, 2 * N], f16, tag="ohcat")
    nc.vector.tensor_scalar(out=ohcat[:, :N], in0=e0b[:], scalar1=iop8f[:],
                            scalar2=None, op0=AOP.is_equal)
    nc.vector.tensor_scalar(out=ohcat[:, N:], in0=e1b[:], scalar1=iop8f[:],
                            scalar2=None, op0=AOP.is_equal)
    ones2n = scan_pool.tile([E, 2 * N], f16, tag="ones2n")
    nc.vector.memset(ones2n[:], 1.0)
    cum = scan_pool.tile([E, 2 * N], f16, tag="cum")
    nc.vector.tensor_tensor_scan(out=cum[:], data0=ones2n[:], data1=ohcat[:],
                                 initial=0.0, op0=AOP.mult, op1=AOP.add)
    ohcum = scan_pool.tile([E, 2 * N], f16, tag="ohcum")
    nc.vector.tensor_tensor(out=ohcum[:], in0=ohcat[:], in1=cum[:],
                            op=AOP.mult)
    ones8 = scan_pool.tile([E, 1], f16, tag="ones8")
    nc.vector.memset(ones8[:], 1.0)
    posrow = scan_pool.tile([1, 2 * N], f32, tag="posrow")

    # ============ F1 body (tiles 2..31) ============
    for ft in range(2, 32):
        f1_block(ft)

    # pos extraction matmuls dropped into the middle of the F1 stream:
    # operands are ready by now, keeps the tail free.
    with tc.tile_pool(name="csps", bufs=2, space="PSUM") as csps:
        for ch in range(2 * N // 512):
            pps = csps.tile([1, 512], f32, tag="pps")
            nc.tensor.matmul(out=pps[:], lhsT=ones8[:],
                             rhs=ohcum[:, ch * 512:(ch + 1) * 512],
                             start=True, stop=True)
            nc.vector.tensor_scalar_add(
                posrow[:, ch * 512:(ch + 1) * 512], pps[:], -1.0)
    nc.gpsimd.dma_start(out=posd[:, :], in_=posrow[:])

    for ft in range(32, FT):
        f1_block(ft)

    # ============ own-token keep/s: gather pos rows {2c+m, 16+2c+m} ======
    f1ps_cm.__exit__(None, None, None)
    w1_cm.__exit__(None, None, None)

    with tc.tile_pool(name="imath", bufs=1) as im:
        iop4 = im.tile([2 * MT, 1], i32, tag="iop4")
        iop4f = im.tile([2 * MT, 1], f32, tag="iop4f")
        nc.gpsimd.iota(iop4[:], pattern=[[0, 1]], base=0, channel_multiplier=1)
        nc.vector.tensor_copy(out=iop4f[:], in_=iop4[:])
        ge2 = im.tile([2 * MT, 1], f32, tag="ge2")
        nc.vector.tensor_scalar(out=ge2[:], in0=iop4f[:], scalar1=float(MT),
                                scalar2=None, op0=AOP.is_ge)
        idxa = im.tile([2 * MT, 1], f32, tag="idxa")
        nc.vector.scalar_tensor_tensor(out=idxa[:], in0=ge2[:],
                                       scalar=float(TT - MT), in1=iop4f[:],
                                       op0=AOP.mult, op1=AOP.add)
        idx4f = im.tile([2 * MT, 1], f32, tag="idx4f")
        nc.vector.scalar_tensor_tensor(out=idx4f[:], in0=cidb4[:],
                                       scalar=float(MT), in1=idxa[:],
                                       op0=AOP.mult, op1=AOP.add)
        idx4i = im.tile([2 * MT, 1], i32, tag="idx4i")
        nc.vector.tensor_copy(out=idx4i[:], in_=idx4f[:])
        pos4 = im.tile([2 * MT, 128], f32, tag="pos4")
        nc.gpsimd.indirect_dma_start(
            out=pos4[:], out_offset=None,
            in_=posd[0:1, :].rearrange("a (r p) -> (a r) p", p=128),
            in_offset=bass.IndirectOffsetOnAxis(ap=idx4i[:, :1], axis=0))
        nc.gpsimd.dma_start(out=posq[:, :], in_=pos4[:])
        nc.gpsimd.dma_start(out=posti[:],
                            in_=posq[:, :].rearrange("r p -> p r"))
        keep = im.tile([128, 2 * MT], f32, tag="keep")
        nc.vector.tensor_scalar(out=keep[:], in0=posti[:], scalar1=float(CAP),
                                scalar2=None, op0=AOP.is_lt)
        for m in range(MT):
            sa = im.tile([128, 1], f32, tag="sa")
            nc.vector.tensor_tensor(out=sa[:], in0=prb[m][:, 0:1],
                                    in1=keep[:, m:m + 1], op=AOP.mult)
            sb = im.tile([128, 1], f32, tag="sb")
            nc.vector.tensor_tensor(out=sb[:], in0=prb[m][:, 1:2],
                                    in1=keep[:, MT + m:MT + m + 1],
                                    op=AOP.mult)
            nc.vector.tensor_tensor(out=sm[m][:], in0=sa[:], in1=sb[:],
                                    op=AOP.add)
        for m in range(MT):
            nc.vector.tensor_scalar_mul(b2s[m][:], b2b[:], sm[m][:, 0:1])

    scan_cm.__exit__(None, None, None)

    # ============ F2: y = g @ w2.T, scaled epilogue ============
    for fc in range(2, 6):
        load_w2(fc)

    with (tc.tile_pool(name="f2ps", bufs=1, space="PSUM") as f2ps,
          tc.tile_pool(name="f2o", bufs=4) as f2o):
        psq = [[f2ps.tile([128, 512], f32, tag=f"f2ps_{m}_{hq}",
                          name=f"f2ps_{m}_{hq}")
                for hq in range(4)] for m in range(MT)]
        for fc in range(FT):
            if fc + 4 < FT:
                load_w2(fc + 4)
            for m in range(MT):
                for hq in range(4):
                    nc.tensor.matmul(
                        out=psq[m][hq][:],
                        lhsT=g[fc][:, m * 128:(m + 1) * 128],
                        rhs=w2t[fc][:, hq * 512:(hq + 1) * 512],
                        start=(fc == 0), stop=(fc == FT - 1))
        for m in range(MT):
            for hq in range(4):
                o_t = f2o.tile([128, 512], f32, tag="o_t")
                nc.vector.scalar_tensor_tensor(
                    out=o_t[:], in0=psq[m][hq][:], scalar=sm[m][:, 0:1],
                    in1=b2s[m][:, hq * 512:(hq + 1) * 512],
                    op0=AOP.mult, op1=AOP.add)
                eng = nc.gpsimd if (hq % 2 == 0) else nc.sync
                eng.dma_start(
                    out=out[m * 128:(m + 1) * 128, hq * 512:(hq + 1) * 512],
                    in_=o_t[:])

    w2_cm.__exit__(None, None, None)
    g_cm.__exit__(None, None, None)


# ======================== host-side glue ========================

_CACHE = {}


def _prep_inputs(hidden_states, router_w, w1, b1, w2, b2):
    x = np.asarray(hidden_states, np.float32).reshape(-1, H)
    xT = np.ascontiguousarray(x.T)                       # [H, N] fp32
    w1Tm = np.asarray(w1, np.float32).T.astype(ml_dtypes.bfloat16)
    w2Tm = np.asarray(w2, np.float32).T.astype(ml_dtypes.bfloat16)
    w1tt = np.ascontiguousarray(
        w1Tm.reshape(HT, 128, FT, 128).transpose(2, 1, 0, 3)).reshape(
            FT, 128, H)
    base = {
        "w1tt": w1tt,
        "rwT": np.ascontiguousarray(np.asarray(router_w, np.float32).T),
        "w2T": np.ascontiguousarray(w2Tm),
        "b1t": np.ascontiguousarray(np.asarray(b1, np.float32).reshape(FT, 128).T),
        "b2r": np.asarray(b2, np.float32).reshape(1, H),
    }
    # router layout: [t_outer, h_inner, h_outer*t_inner] fp32
    xtmf = np.ascontiguousarray(
        xT.reshape(HT, 128, TT, 128).transpose(2, 1, 0, 3)).reshape(TT, 128, H)
    # FFN rhs layout: xT in [h_outer, h_inner, token] bf16, per-core slice
    xTb = xT.astype(ml_dtypes.bfloat16)
    ins = []
    for c in range(NCORES):
        m = dict(base)
        m["xtm"] = np.ascontiguousarray(xtmf[MT * c:MT * (c + 1)])
        m["xtc"] = np.ascontiguousarray(
            xTb[:, c * TOKC:(c + 1) * TOKC].reshape(HT, 128, TOKC)
            .transpose(1, 0, 2).reshape(128, HT * TOKC))
        m["cid"] = np.full((1, 1), float(c), np.float32)
        ins.append(m)
    return ins


def _get_nc():
    if "nc" not in _CACHE:
        apply()  # tile drain patch
        nc = bass.Bass(num_devices=NCORES)
        build_moe(nc)
        split_multi_waits(nc)
        _CACHE["nc"] = nc
    return _CACHE["nc"]


def kernel(hidden_states, router_w, w1, b1, w2, b2):
    from concourse.bass_utils import run_bass_kernel_spmd

    orig_shape = np.asarray(hidden_states).shape
    nc = _get_nc()
    ins = _prep_inputs(hidden_states, router_w, w1, b1, w2, b2)
    res = run_bass_kernel_spmd(nc, ins, core_ids=list(range(NCORES)))
    full = np.concatenate([res.results[c]["out"] for c in range(NCORES)], axis=0)
    return full.reshape(orig_shape).astype(np.float32)


# revision 21
# speedup vs baseline: 2.6778x; 1.0777x over previous
"""Trainium2 8-core MoE layer kernel (collapsed shared-expert FFN, Bass/Tile).

The reference MoE applies the SAME w1/b1/w2/b2 to every expert's slice of
the dispatch buffer, so the whole layer collapses algebraically to

    out[t] = s(t) * (gelu(x[t] @ w1.T + b1) @ w2.T + b2)
    s(t)   = p0(t)*keep0(t) + p1(t)*keep1(t)

where keep_k(t) is the capacity-survival flag from the slot-major
cumulative-count over the global top-2 assignment sequence.  Only s(t)
needs global routing info; the FFN itself is a dense per-token FFN.

Sharding: token-parallel, 256 tokens per core.  Each core runs the
fp32 router on its own tokens, AllGathers the 4KB of routing decisions,
recomputes the global cumsum locally (replicated, deterministic), and
runs the dense FFN on its 256 tokens with w1/w2 streamed from HBM in
bf16.  The routing/scan chain runs on vector/scalar/gpsimd engines
concurrently with the F1 matmul stream; s(t) is only consumed by the
final epilogue scaling.
"""
import numpy as np
import ml_dtypes

import concourse.bass as bass
import concourse.mybir as mybir
import concourse.tile as tile

_PATCH_DOC = """Patch TileContext._drain_and_barrier: the stock version stuffs every
outstanding semaphore wait onto one SP Drain instruction; the installed
walrus rejects >1 sync wait per non-EventSemaphore instruction
("Too many sync wait commands"). Split the waits across a chain of SP
nops, then drain/barrier as before."""
import concourse.tile as tile_mod
from concourse.vector_clock import ScopedClock


def _patched_drain_and_barrier(self, tick_clock, wait_clock):
    nc = self.nc
    carrier = nc.sync.nop(nofuse=True, hint="drain_wait_carrier")
    wait_clock.add_sem_waits(
        carrier.ins, ScopedClock({None: tick_clock.global_clock})
    )
    waits = list(carrier.ins.sync_info.on_wait)
    if len(waits) > 1:
        carrier.ins.sync_info.on_wait = waits[:1]
        import bass_rust as _br
        for w in waits[1:]:
            extra = nc.sync.nop(nofuse=True, hint="drain_wait_carrier")
            extra.ins.sync_info = _br.SyncInfo(on_wait=[w], on_update=[])

    nc.sync.drain()
    nc.all_engine_barrier()
    assert self.sems is not None
    popped = nc._tile_sem_poison_stack.pop()
    assert popped is self._sem_poison
    nc.clear_and_free_semaphores(list(self.sems.allocated().values()))
    nc.all_engine_barrier()


def apply():
    tile_mod.TileContext._drain_and_barrier = _patched_drain_and_barrier


import concourse.mybir as mybir
import bass_rust as _br


def split_multi_waits(nc):
    """Walrus in this container accepts at most ONE sync wait per
    instruction. Hoist extra waits onto same-engine NoOps inserted
    immediately before the offending instruction."""
    ctr = 0
    for f in nc.m.functions:
        for b in f.blocks:
            insts = b.instructions
            need = any(
                inst.sync_info is not None and len(inst.sync_info.on_wait) > 1
                for inst in insts
            )
            if not need:
                continue
            out = []
            for inst in insts:
                si = inst.sync_info
                if si is not None and len(si.on_wait) > 1:
                    waits = list(si.on_wait)
                    for w in waits[:-1]:
                        nop = mybir.InstNoOp(name=f"I-wsplit-{ctr}", ins=[], outs=[])
                        ctr += 1
                        nop.engine = inst.engine
                        nop.sync_info = _br.SyncInfo(on_wait=[w], on_update=[])
                        out.append(nop)
                    si.on_wait = waits[-1:]
                out.append(inst)
            b.instructions = out
    return ctr


E, TOPK, CAP, H, F, N, NCORES = 8, 2, 512, 2048, 8192, 2048, 8
TT = N // 128                # 16 token tiles (global)
HT = H // 128                # 16 hidden tiles
FT = F // 128                # 64 ffn tiles
TOKC = N // NCORES           # 256 tokens per core
MT = TOKC // 128             # 2 local token tiles

f32 = mybir.dt.float32
f16 = mybir.dt.float16
bf16 = mybir.dt.bfloat16
i32 = mybir.dt.int32
AOP = mybir.AluOpType
AFT = mybir.ActivationFunctionType
AX = mybir.AxisListType


def build_moe(nc: bass.Bass):
    xtm = nc.dram_tensor("xtm", [MT, 128, H], f32, kind="ExternalInput")
    xtc = nc.dram_tensor("xtc", [128, HT * TOKC], bf16, kind="ExternalInput")
    rwT = nc.dram_tensor("rwT", [H, E], f32, kind="ExternalInput")
    w1T = nc.dram_tensor("w1tt", [FT, 128, HT * 128], bf16, kind="ExternalInput")
    w2T = nc.dram_tensor("w2T", [F, H], bf16, kind="ExternalInput")
    b1t = nc.dram_tensor("b1t", [128, FT], f32, kind="ExternalInput")
    b2r = nc.dram_tensor("b2r", [1, H], f32, kind="ExternalInput")
    cid = nc.dram_tensor("cid", [1, 1], f32, kind="ExternalInput")
    out = nc.dram_tensor("out", [TOKC, H], f32, kind="ExternalOutput")

    rloc = nc.dram_tensor("rloc", [2, TOKC], f32)
    rall = nc.dram_tensor("rall", [2 * NCORES, TOKC], f32, addr_space="Shared")
    posd = nc.dram_tensor("posd", [1, 2 * N], f32)
    posq = nc.dram_tensor("posq", [2 * MT, 128], f32)

    with tile.TileContext(nc, num_cores=NCORES) as tc:
        with tc.tile_pool(name="persist", bufs=1) as persist:
            _body(nc, tc, persist, xtm, xtc, rwT, w1T, w2T, b1t, b2r, cid,
                  out, rloc, rall, posd, posq)
    return nc


def _body(nc, tc, persist, xtm, xtc, rwT, w1T, w2T, b1t, b2r, cid, out,
          rloc, rall, posd, posq):
    # ---- persistent tiles ----
    xts = persist.tile([128, HT * TOKC], bf16, tag="xts")      # xT own tokens
    b2b = persist.tile([128, H], f32, tag="b2b")
    b2s = [persist.tile([128, H], f32, tag=f"b2s_{m}", name=f"b2s_{m}")
           for m in range(MT)]
    cidb4 = persist.tile([2 * MT, 1], f32, tag="cidb4")
    b1sb = persist.tile([128, FT], f32, tag="b1sb")
    rws = persist.tile([128, HT * E], f32, tag="rws")
    sm = [persist.tile([128, 1], f32, tag=f"sm_{m}", name=f"sm_{m}")
          for m in range(MT)]
    prb = [persist.tile([128, 2], f32, tag=f"prb_{m}", name=f"prb_{m}")
           for m in range(MT)]
    lsb2 = [persist.tile([128, E], f32, tag=f"lsb2_{m}", name=f"lsb2_{m}")
            for m in range(MT)]
    nm2 = [persist.tile([128, 1], f32, tag=f"nm2_{m}", name=f"nm2_{m}")
           for m in range(MT)]
    posti = persist.tile([128, 2 * MT], f32, tag="posti")

    # pools in LIFO lifetime order (longest-lived entered first)
    g_cm = tc.tile_pool(name="g", bufs=1)
    g_pool = g_cm.__enter__()
    w2_cm = tc.tile_pool(name="w2s", bufs=6)
    w2_pool = w2_cm.__enter__()
    scan_cm = tc.tile_pool(name="scan", bufs=1)
    scan_pool = scan_cm.__enter__()
    w1_cm = tc.tile_pool(name="w1s", bufs=8)
    w1_pool = w1_cm.__enter__()
    f1ps_cm = tc.tile_pool(name="f1ps", bufs=6, space="PSUM")
    f1ps_pool = f1ps_cm.__enter__()

    # initial small loads: gpsimd handles router-critical, scalar the rest
    nc.gpsimd.dma_start(out=cidb4[:],
                        in_=cid[0:1, :].partition_broadcast(2 * MT).opt())
    nc.gpsimd.dma_start(out=rws[:].rearrange("p (c e) -> p c e", e=E),
                        in_=rwT[:, :].rearrange("(c p) e -> p c e", p=128))
    nc.scalar.dma_start(out=xts[:], in_=xtc[:, :])

    w1t = {}

    def load_w1(ft):
        t = w1_pool.tile([128, HT * 128], bf16, tag="w1t")
        w1t[ft] = t
        eng = nc.sync if (ft % 2 == 0) else nc.scalar
        eng.dma_start(out=t[:], in_=w1T[ft, :, :])

    for ft in range(4):
        load_w1(ft)
    nc.scalar.dma_start(out=b1sb[:], in_=b1t[:, :])
    load_w1(4)
    load_w1(5)
    nc.scalar.dma_start(out=b2b[:], in_=b2r[0:1, :].partition_broadcast(128).opt())

    g = [g_pool.tile([128, TOKC], bf16, tag=f"g_{ft}", name=f"g_{ft}")
         for ft in range(FT)]

    def f1_block(ft):
        if ft + 6 < FT:
            load_w1(ft + 6)
        ps = f1ps_pool.tile([128, TOKC], f32, tag="f1ps")
        for hc in range(HT):
            nc.tensor.matmul(
                out=ps[:], lhsT=w1t[ft][:, hc * 128:(hc + 1) * 128],
                rhs=xts[:, hc * TOKC:(hc + 1) * TOKC],
                start=(hc == 0), stop=(hc == HT - 1))
        nc.scalar.activation(out=g[ft][:], in_=ps[:], func=AFT.Gelu,
                             bias=b1sb[:, ft:ft + 1], scale=1.0)

    # F1 head: two tiles before the router matmuls so the PE queue never
    # waits on the (slightly later) xtm DMA.
    f1_block(0)
    f1_block(1)

    # ============ Router (own 256 tokens, fp32) + AllGather ============
    # Only the top-2 INDICES gate the AllGather (softmax is monotonic);
    # probabilities are computed after F1, overlapped with F2.
    with (tc.tile_pool(name="r_x", bufs=2) as r_x,
          tc.tile_pool(name="r_ps", bufs=2, space="PSUM") as r_ps,
          tc.tile_pool(name="r_sb", bufs=2) as r_sb):
        xtt = []
        for tt2 in range(MT):
            xt_t = r_x.tile([128, H], f32, tag="xt_t", name=f"xt_t{tt2}")
            nc.gpsimd.dma_start(out=xt_t[:], in_=xtm[tt2, :, :])
            xtt.append(xt_t)
        for tt2 in range(MT):
            ps = r_ps.tile([128, E], f32, tag="r_ps")
            for hc in range(HT):
                nc.tensor.matmul(
                    out=ps[:], lhsT=xtt[tt2][:, hc * 128:(hc + 1) * 128],
                    rhs=rws[:, hc * E:(hc + 1) * E],
                    start=(hc == 0), stop=(hc == HT - 1))
            nc.vector.tensor_copy(out=lsb2[tt2][:], in_=ps[:])
            mx = r_sb.tile([128, 1], f32, tag="mx")
            nc.vector.tensor_reduce(out=mx[:], in_=lsb2[tt2][:], op=AOP.max,
                                    axis=AX.X)
            nc.vector.tensor_scalar_mul(nm2[tt2][:], mx[:], -1.0)
            mx8 = r_sb.tile([128, 8], f32, tag="mx8")
            ix8 = r_sb.tile([128, 8], mybir.dt.uint32, tag="ix8")
            nc.vector.max_with_indices(out_max=mx8[:], out_indices=ix8[:],
                                       in_=lsb2[tt2][:])
            ev = r_sb.tile([128, 2], f32, tag="ev")
            nc.vector.tensor_copy(out=ev[:, 0:1], in_=ix8[:, 0:1])
            nc.vector.tensor_copy(out=ev[:, 1:2], in_=ix8[:, 1:2])
            nc.gpsimd.dma_start(
                out=rloc[:, tt2 * 128:(tt2 + 1) * 128].rearrange("a t -> t a"),
                in_=ev[:])
        nc.gpsimd.collective_compute(
            "AllGather", AOP.bypass,
            replica_groups=[list(range(NCORES))],
            ins=[rloc[:, :].opt()],
            outs=[rall[:, :].opt()])

    # w2 prefetch for the F1->F2 boundary (gpsimd queue, right after AG)
    w2t = {}

    def load_w2(fc, eng=None):
        t = w2_pool.tile([128, H], bf16, tag="w2t")
        w2t[fc] = t
        if eng is None:
            eng = nc.sync if (fc % 2 == 0) else nc.scalar
        eng.dma_start(out=t[:], in_=w2T[fc * 128:(fc + 1) * 128, :])

    load_w2(0, eng=nc.gpsimd)
    load_w2(1, eng=nc.gpsimd)

    # ============ Scan: global slot-major position per assignment ======
    # (vector engine; runs concurrently with F1 matmuls)
    iop8 = scan_pool.tile([E, 1], i32, tag="iop8")
    iop8f = scan_pool.tile([E, 1], f32, tag="iop8f")
    nc.gpsimd.iota(iop8[:], pattern=[[0, 1]], base=0, channel_multiplier=1)
    nc.vector.tensor_copy(out=iop8f[:], in_=iop8[:])
    e0b = scan_pool.tile([E, N], f16, tag="e0b")
    e1b = scan_pool.tile([E, N], f16, tag="e1b")
    rallv = rall[:, :].rearrange("(c a) t -> a c t", a=2)
    nc.gpsimd.dma_start(
        out=e0b[:].rearrange("p (c t) -> p c t", t=TOKC),
        in_=rallv[0:1, :, :].partition_broadcast(E).opt())
    nc.gpsimd.dma_start(
        out=e1b[:].rearrange("p (c t) -> p c t", t=TOKC),
        in_=rallv[1:2, :, :].partition_broadcast(E).opt())
    ohcat = scan_pool.tile([E, 2 * N], f16, tag="ohcat")
    nc.vector.tensor_scalar(out=ohcat[:, :N], in0=e0b[:], scalar1=iop8f[:],
                            scalar2=None, op0=AOP.is_equal)
    nc.vector.tensor_scalar(out=ohcat[:, N:], in0=e1b[:], scalar1=iop8f[:],
                            scalar2=None, op0=AOP.is_equal)
    ones2n = scan_pool.tile([E, 2 * N], f16, tag="ones2n")
    nc.vector.memset(ones2n[:], 1.0)
    cum = scan_pool.tile([E, 2 * N], f16, tag="cum")
    nc.vector.tensor_tensor_scan(out=cum[:], data0=ones2n[:], data1=ohcat[:],
                                 initial=0.0, op0=AOP.mult, op1=AOP.add)
    ohcum = scan_pool.tile([E, 2 * N], f16, tag="ohcum")
    nc.vector.tensor_tensor(out=ohcum[:], in0=ohcat[:], in1=cum[:],
                            op=AOP.mult)
    ones8 = scan_pool.tile([E, 1], f16, tag="ones8")
    nc.vector.memset(ones8[:], 1.0)
    posrow = scan_pool.tile([1, 2 * N], f32, tag="posrow")

    # ============ F1 body (tiles 2..63) ============
    for ft in range(2, FT):
        f1_block(ft)

    # pos extraction matmuls after the F1 stream (the AllGather + scan
    # complete well before F1 ends, so these never stall the PE).
    with tc.tile_pool(name="csps", bufs=2, space="PSUM") as csps:
        for ch in range(2 * N // 512):
            pps = csps.tile([1, 512], f32, tag="pps")
            nc.tensor.matmul(out=pps[:], lhsT=ones8[:],
                             rhs=ohcum[:, ch * 512:(ch + 1) * 512],
                             start=True, stop=True)
            nc.vector.tensor_scalar_add(
                posrow[:, ch * 512:(ch + 1) * 512], pps[:], -1.0)
    nc.gpsimd.dma_start(out=posd[:, :], in_=posrow[:])

    # ============ own-token keep/s: gather pos rows {2c+m, 16+2c+m} ======
    f1ps_cm.__exit__(None, None, None)
    w1_cm.__exit__(None, None, None)

    with tc.tile_pool(name="imath", bufs=1) as im:
        # late softmax: probabilities for own tokens (overlaps F2)
        for m in range(MT):
            ex = im.tile([128, E], f32, tag="ex", name=f"ex_{m}")
            ssum = im.tile([128, 1], f32, tag="ssum", name=f"ssum_{m}")
            nc.scalar.activation(out=ex[:], in_=lsb2[m][:], func=AFT.Exp,
                                 bias=nm2[m][:], scale=1.0, accum_out=ssum[:])
            rcp = im.tile([128, 1], f32, tag="rcp", name=f"rcp_{m}")
            nc.vector.reciprocal(out=rcp[:], in_=ssum[:])
            pr = im.tile([128, E], f32, tag="pr", name=f"pr_{m}")
            nc.vector.tensor_scalar_mul(pr[:], ex[:], rcp[:])
            mx8 = im.tile([128, 8], f32, tag="mx8", name=f"mx8_{m}")
            ix8d = im.tile([128, 8], mybir.dt.uint32, tag="ix8d",
                           name=f"ix8d_{m}")
            nc.vector.max_with_indices(out_max=mx8[:], out_indices=ix8d[:],
                                       in_=pr[:])
            nc.vector.tensor_copy(out=prb[m][:], in_=mx8[:, 0:2])

        iop4 = im.tile([2 * MT, 1], i32, tag="iop4")
        iop4f = im.tile([2 * MT, 1], f32, tag="iop4f")
        nc.gpsimd.iota(iop4[:], pattern=[[0, 1]], base=0, channel_multiplier=1)
        nc.vector.tensor_copy(out=iop4f[:], in_=iop4[:])
        ge2 = im.tile([2 * MT, 1], f32, tag="ge2")
        nc.vector.tensor_scalar(out=ge2[:], in0=iop4f[:], scalar1=float(MT),
                                scalar2=None, op0=AOP.is_ge)
        idxa = im.tile([2 * MT, 1], f32, tag="idxa")
        nc.vector.scalar_tensor_tensor(out=idxa[:], in0=ge2[:],
                                       scalar=float(TT - MT), in1=iop4f[:],
                                       op0=AOP.mult, op1=AOP.add)
        idx4f = im.tile([2 * MT, 1], f32, tag="idx4f")
        nc.vector.scalar_tensor_tensor(out=idx4f[:], in0=cidb4[:],
                                       scalar=float(MT), in1=idxa[:],
                                       op0=AOP.mult, op1=AOP.add)
        idx4i = im.tile([2 * MT, 1], i32, tag="idx4i")
        nc.vector.tensor_copy(out=idx4i[:], in_=idx4f[:])
        pos4 = im.tile([2 * MT, 128], f32, tag="pos4")
        nc.gpsimd.indirect_dma_start(
            out=pos4[:], out_offset=None,
            in_=posd[0:1, :].rearrange("a (r p) -> (a r) p", p=128),
            in_offset=bass.IndirectOffsetOnAxis(ap=idx4i[:, :1], axis=0))
        nc.gpsimd.dma_start(out=posq[:, :], in_=pos4[:])
        nc.gpsimd.dma_start(out=posti[:],
                            in_=posq[:, :].rearrange("r p -> p r"))
        keep = im.tile([128, 2 * MT], f32, tag="keep")
        nc.vector.tensor_scalar(out=keep[:], in0=posti[:], scalar1=float(CAP),
                                scalar2=None, op0=AOP.is_lt)
        for m in range(MT):
            sa = im.tile([128, 1], f32, tag="sa")
            nc.vector.tensor_tensor(out=sa[:], in0=prb[m][:, 0:1],
                                    in1=keep[:, m:m + 1], op=AOP.mult)
            sb = im.tile([128, 1], f32, tag="sb")
            nc.vector.tensor_tensor(out=sb[:], in0=prb[m][:, 1:2],
                                    in1=keep[:, MT + m:MT + m + 1],
                                    op=AOP.mult)
            nc.vector.tensor_tensor(out=sm[m][:], in0=sa[:], in1=sb[:],
                                    op=AOP.add)
        for m in range(MT):
            nc.vector.tensor_scalar_mul(b2s[m][:], b2b[:], sm[m][:, 0:1])

    scan_cm.__exit__(None, None, None)

    # ============ F2: y = g @ w2.T, scaled epilogue ============
    for fc in range(2, 6):
        load_w2(fc)

    with (tc.tile_pool(name="f2ps", bufs=1, space="PSUM") as f2ps,
          tc.tile_pool(name="f2o", bufs=4) as f2o):
        psq = [[f2ps.tile([128, 512], f32, tag=f"f2ps_{m}_{hq}",
                          name=f"f2ps_{m}_{hq}")
                for hq in range(4)] for m in range(MT)]
        for fc in range(FT):
            if fc + 4 < FT:
                load_w2(fc + 4)
            for m in range(MT):
                for hq in range(4):
                    nc.tensor.matmul(
                        out=psq[m][hq][:],
                        lhsT=g[fc][:, m * 128:(m + 1) * 128],
                        rhs=w2t[fc][:, hq * 512:(hq + 1) * 512],
                        start=(fc == 0), stop=(fc == FT - 1))
        for m in range(MT):
            for hq in range(4):
                o_t = f2o.tile([128, 512], f32, tag="o_t")
                nc.vector.scalar_tensor_tensor(
                    out=o_t[:], in0=psq[m][hq][:], scalar=sm[m][:, 0:1],
                    in1=b2s[m][:, hq * 512:(hq + 1) * 512],
                    op0=AOP.mult, op1=AOP.add)
                eng = nc.gpsimd if (hq % 2 == 0) else nc.sync
                eng.dma_start(
                    out=out[m * 128:(m + 1) * 128, hq * 512:(hq + 1) * 512],
                    in_=o_t[:])

    w2_cm.__exit__(None, None, None)
    g_cm.__exit__(None, None, None)


# ======================== host-side glue ========================

_CACHE = {}


def _prep_inputs(hidden_states, router_w, w1, b1, w2, b2):
    x = np.asarray(hidden_states, np.float32).reshape(-1, H)
    xT = np.ascontiguousarray(x.T)                       # [H, N] fp32
    w1Tm = np.asarray(w1, np.float32).T.astype(ml_dtypes.bfloat16)
    w2Tm = np.asarray(w2, np.float32).T.astype(ml_dtypes.bfloat16)
    w1tt = np.ascontiguousarray(
        w1Tm.reshape(HT, 128, FT, 128).transpose(2, 1, 0, 3)).reshape(
            FT, 128, H)
    base = {
        "w1tt": w1tt,
        "rwT": np.ascontiguousarray(np.asarray(router_w, np.float32).T),
        "w2T": np.ascontiguousarray(w2Tm),
        "b1t": np.ascontiguousarray(np.asarray(b1, np.float32).reshape(FT, 128).T),
        "b2r": np.asarray(b2, np.float32).reshape(1, H),
    }
    # router layout: [t_outer, h_inner, h_outer*t_inner] fp32
    xtmf = np.ascontiguousarray(
        xT.reshape(HT, 128, TT, 128).transpose(2, 1, 0, 3)).reshape(TT, 128, H)
    # FFN rhs layout: xT in [h_outer, h_inner, token] bf16, per-core slice
    xTb = xT.astype(ml_dtypes.bfloat16)
    ins = []
    for c in range(NCORES):
        m = dict(base)
        m["xtm"] = np.ascontiguousarray(xtmf[MT * c:MT * (c + 1)])
        m["xtc"] = np.ascontiguousarray(
            xTb[:, c * TOKC:(c + 1) * TOKC].reshape(HT, 128, TOKC)
            .transpose(1, 0, 2).reshape(128, HT * TOKC))
        m["cid"] = np.full((1, 1), float(c), np.float32)
        ins.append(m)
    return ins


def _get_nc():
    if "nc" not in _CACHE:
        apply()  # tile drain patch
        nc = bass.Bass(num_devices=NCORES)
        build_moe(nc)
        split_multi_waits(nc)
        _CACHE["nc"] = nc
    return _CACHE["nc"]


def kernel(hidden_states, router_w, w1, b1, w2, b2):
    from concourse.bass_utils import run_bass_kernel_spmd

    orig_shape = np.asarray(hidden_states).shape
    nc = _get_nc()
    ins = _prep_inputs(hidden_states, router_w, w1, b1, w2, b2)
    res = run_bass_kernel_spmd(nc, ins, core_ids=list(range(NCORES)))
    full = np.concatenate([res.results[c]["out"] for c in range(NCORES)], axis=0)
    return full.reshape(orig_shape).astype(np.float32)
